# revision 4
# baseline (speedup 1.0000x reference)
"""CEMA kernel for Trainium2: batch-mean + EMA scan over sequence.

Computes, for x[B=8, S=4096, D=2048] fp32:
    m = mean(x, axis=0)                       # [S, D]
    ema_t = a*ema_{t-1} + (1-a)*m_t  (scan)   # [S, D]
    out = broadcast(ema, [B, S, D])

Distribution: the EMA scan is elementwise in D, so D is sharded across the
8 cores (DC=256 columns each) — no collectives needed.

Per-core algorithm: the sequence is cut into NBLK=33 scan blocks of L=127
steps (tail 32). The batch sum per block is a 3-level in-place halving
tree on DVE. The scan itself is two PE matmuls per block into one PSUM:
    ps[i] (i>=1) = ema at step t0+i-1, ps[0] duplicates the last step so
    the next block's carry is read from PSUM partition 0:
      mm_data : lhsT_d[j,i] = a^(i-1-j)*(1-a)/B  (k=127, off carry chain)
      mm_carry: lhsT_c[0,i] = a^i                (k=1, rank-1 carry term)
    carry handoff = same-partition ACT copy ps[0:1] -> [1,DC] tile.

DMA strategy (the perf-critical part):
  * SWDGE (nc.gpsimd) only for bulk transfers — HWDGE (nc.sync) put every
    descriptor of this pattern on ONE SDMA engine (~26 GB/s measured).
  * SWDGE per-descriptor overhead is large (~19 tiny ring packets per
    descriptor), so descriptor count is minimized via a host-side blocked
    transpose: xh[p, n*2048:(n+1)*2048] = row n*127+p of the [S, B*DC]
    slab. Loads of G=8 blocks are then ONE op with 127 descriptors of
    64KB contiguous each. Output accumulates in SBUF ([128, 33*256]) and
    is stored with 2 ops at the end.
"""

import sys

for _p in ("/opt/trn_rl_repo", "/root/.axon_site/_ro/trn_rl_repo"):
    if _p not in sys.path:
        sys.path.append(_p)

import numpy as np

import concourse.bass as bass  # noqa: F401  (AP helpers)
import concourse.tile as tile
from concourse import bacc, mybir
from concourse import bass_utils

ALPHA = 0.99
B, S, D = 8, 4096, 2048
NCORES = 8
DC = D // NCORES          # 256 columns per core
L = 127                   # scan-block length (PSUM: 127 emas + 1 dup row)
NBLK = (S + L - 1) // L   # 33 (32 full + tail of 32)
G = 8                     # blocks per load DMA
SP = NBLK * L             # padded sequence length (4191)
F32 = mybir.dt.float32


def _make_lhsT() -> tuple[np.ndarray, np.ndarray]:
    """(lhsT_d [127,128], lhsT_c [1,128]) for out[i,d]=sum_k lhsT[k,i]rhs[k,d].

    ps row i (i>=1) = ema_{t0+i-1} = a^i*carry + sum_j a^(i-1-j)*scale*S_j;
    row 0 duplicates row 127 so the next carry lands on PSUM partition 0.
    """
    scale = (1.0 - ALPHA) / B
    d = np.zeros((L, 128), dtype=np.float64)
    c = np.zeros((1, 128), dtype=np.float64)
    for i in range(1, 128):
        c[0, i] = ALPHA ** i
        for j in range(i):
            d[j, i] = ALPHA ** (i - 1 - j) * scale
    d[:, 0] = d[:, 127]
    c[0, 0] = c[0, 127]
    return d.astype(np.float32), c.astype(np.float32)


def build_nc():
    nc = bacc.Bacc(
        "TRN2", target_bir_lowering=False, debug=False, enable_asserts=False
    )
    xh = nc.dram_tensor("xh", [L, NBLK * B * DC], F32, kind="ExternalInput").ap()
    td = nc.dram_tensor("td", [L, 128], F32, kind="ExternalInput").ap()
    tcr = nc.dram_tensor("tc", [1, 128], F32, kind="ExternalInput").ap()
    yh = nc.dram_tensor("yh", [L, NBLK * DC], F32, kind="ExternalOutput").ap()

    BDC = B * DC  # 2048
    with tile.TileContext(nc) as tc:
        with (
            tc.tile_pool(name="const", bufs=1) as const_pool,
            tc.tile_pool(name="xs", bufs=2) as xs_pool,
            tc.tile_pool(name="psum", bufs=4, space="PSUM") as psum_pool,
            tc.tile_pool(name="carry", bufs=2) as c_pool,
            tc.tile_pool(name="yacc", bufs=1) as y_pool,
        ):
            td_sb = const_pool.tile([L, 128], F32)
            nc.sync.dma_start(td_sb[:, :], td)
            tc_sb = const_pool.tile([1, 128], F32)
            nc.sync.dma_start(tc_sb[:, :], tcr)
            ya = y_pool.tile([128, NBLK * DC], F32)

            cprev = None
            n = 0
            for g in range((NBLK + G - 1) // G):
                nb = min(G, NBLK - g * G)
                xt = xs_pool.tile([128, G * BDC], F32)
                nc.gpsimd.dma_start(
                    xt[0:L, 0 : nb * BDC],
                    xh[:, g * G * BDC : (g * G + nb) * BDC],
                )
                for l in range(nb):
                    c0 = l * BDC
                    k = L if n < NBLK - 1 else S - (NBLK - 1) * L
                    # batch sum: halving tree over the b-major free axis
                    w = BDC
                    while w > DC:
                        h = w // 2
                        nc.vector.tensor_add(
                            xt[0:L, c0 : c0 + h],
                            xt[0:L, c0 : c0 + h],
                            xt[0:L, c0 + h : c0 + w],
                        )
                        w = h
                    ps = psum_pool.tile([128, DC], F32)
                    if cprev is None:
                        nc.tensor.matmul(
                            ps[:, :], td_sb[0:k, :], xt[0:k, c0 : c0 + DC],
                            start=True, stop=True,
                        )
                    else:
                        nc.tensor.matmul(
                            ps[:, :], td_sb[0:k, :], xt[0:k, c0 : c0 + DC],
                            start=True, stop=False,
                        )
                        nc.tensor.matmul(
                            ps[:, :], tc_sb[0:1, :], cprev[0:1, :],
                            start=False, stop=True,
                        )
                    if n < NBLK - 1:
                        cn = c_pool.tile([1, DC], F32)
                        nc.scalar.copy(cn[0:1, :], ps[0:1, 0:DC])
                        cprev = cn
                    nc.vector.tensor_copy(ya[:, n * DC : (n + 1) * DC], ps[:, :])
                    n += 1
                if g == 1:
                    # blocks 0..15 traced: store first half, overlapped
                    nc.gpsimd.dma_start(
                        yh[:, 0 : 16 * DC], ya[1:128, 0 : 16 * DC]
                    )
            nc.gpsimd.dma_start(
                yh[:, 16 * DC : NBLK * DC], ya[1:128, 16 * DC : NBLK * DC]
            )
    nc.compile()
    return nc


_NC_CACHE = None


def _get_nc():
    global _NC_CACHE
    if _NC_CACHE is None:
        _NC_CACHE = build_nc()
    return _NC_CACHE


def make_in_maps(x: np.ndarray) -> list[dict]:
    x = np.asarray(x, dtype=np.float32)
    td_np, tc_np = _make_lhsT()
    in_maps = []
    for i in range(NCORES):
        slab = x[:, :, i * DC : (i + 1) * DC]  # [B, S, DC]
        xs2d = slab.transpose(1, 0, 2).reshape(S, B * DC)
        xp = np.zeros((SP, B * DC), dtype=np.float32)
        xp[:S] = xs2d
        # blocked transpose: xh[p, n*2048:(n+1)*2048] = xp[n*127 + p]
        xh = np.ascontiguousarray(
            xp.reshape(NBLK, L, B * DC).transpose(1, 0, 2)
        ).reshape(L, NBLK * B * DC)
        in_maps.append({"xh": xh, "td": td_np, "tc": tc_np})
    return in_maps


def run(x: np.ndarray, trace: bool = False, **kw):
    """Returns (out [B,S,D] fp32, BassKernelResults)."""
    nc = _get_nc()
    res = bass_utils.run_bass_kernel_spmd(
        nc, make_in_maps(x), core_ids=list(range(NCORES)), trace=trace, **kw
    )
    cores = []
    for r in res.results:
        yh = r["yh"]  # [127, NBLK*DC]
        em = yh.reshape(L, NBLK, DC).transpose(1, 0, 2).reshape(SP, DC)[:S]
        cores.append(em)
    emas = np.concatenate(cores, axis=1)  # [S, D]
    out = np.broadcast_to(emas[None, :, :], (B, S, D))
    return out, res


def kernel(x: np.ndarray) -> np.ndarray:
    out, _ = run(x, trace=False)
    return out
